# revision 10
# baseline (speedup 1.0000x reference)
"""HardTripletLoss on 8 Trainium2 NeuronCores (Bass/Tile).

Math
----
reference: emb = l2_normalize(embeddings); dist = cdist(emb, emb);
  pos_stat[i] = mean_{j: same class, j!=i} dist[i,j]
  neg_stat[i] = min_{j: diff class} dist[i,j]
  loss = mean over valid rows of relu(pos_stat - neg_stat + 1)

For unit vectors dist^2 = 2 - 2*ghat where ghat = N @ N.T.  We fold the
class mask into the GEMM itself: with Y = onehot(labels) [64, B],

  A = [ N.T ; +sqrt(2)*Y ; -sqrt(2)*Y ]   (640 x 4096, zero-padded rows)

and contracting lhsT = [N.T ; +sqrt(2)*Y] (the +Y block from a separate
small per-core tile) against rhs = [N.T ; -sqrt(2)*Y] gives
P = ghat - 2*S  (S = same-class indicator incl diagonal).  Then per row:
  masked positive dists = sqrt(2*relu(-1 - P))   (diff-class & diagonal -> 0)
  hardest negative      = sqrt(relu(2 - 2*rowmax(P)))
      (rowmax(P) = max over diff-class ghat, since same-class P <= -1+eps)

Sharding: rows split 512/core (data parallel).  Every core holds all 4096
columns of A in SBUF (10.5 MB) as 8 slabs of 512 columns; slab order is
rotated per core so each core's first-loaded slab contains its own shard
columns (the matmul stationary operand), letting the GEMM start after the
first 1.3 MB DMA.  Row stats are order-invariant (sum/max over columns).

Host does only input marshaling (normalize+transpose+onehot packing,
O(B*D), 0.02% of the FLOPs) and the final scalar mean over 4096
device-computed per-row losses.
"""

import sys

if "/opt/trn_rl_repo" not in sys.path:
    sys.path.insert(0, "/opt/trn_rl_repo")

import numpy as np

import concourse.bass as bass
import concourse.bacc as bacc
import concourse.mybir as mybir
import concourse.tile as tile
from concourse.bass_utils import run_bass_kernel_spmd

F32 = mybir.dt.float32
ALU = mybir.AluOpType
ACTF = mybir.ActivationFunctionType
AXX = mybir.AxisListType.X

B = 4096
D = 512
C = 64
NCORES = 8
SHARD = B // NCORES          # 512 rows per core
MT = SHARD // 128            # 4 m-tiles per core
NJ = 8                       # column slabs of 512
KC = 5                       # k-chunks of 128 (4 data + 1 +/- onehot)
SLABW = KC * 512             # 2560
GW = 2                       # slabs per psum group ([128, 1024] = 2 banks)
NG = NJ // GW                # 4 psum groups

MARGIN = 1.0


def _build_nc():
    nc = bacc.Bacc(
        "TRN2",
        target_bir_lowering=False,
        debug=False,
        enable_asserts=False,
        num_devices=NCORES,
    )
    atp = nc.dram_tensor("atp", [NJ, 128, SLABW], F32, kind="ExternalInput")
    yl = nc.dram_tensor("yl", [C, SHARD], F32, kind="ExternalInput")
    aux = nc.dram_tensor("aux", [128, 2 * MT], F32, kind="ExternalInput")
    outp = nc.dram_tensor("outp", [128, MT], F32, kind="ExternalOutput")

    with tile.TileContext(nc) as tc:
        with (
            tc.tile_pool(name="slabs", bufs=1) as slabs,
            tc.tile_pool(name="psum", bufs=4, space=bass.MemorySpace.PSUM) as psum,
            tc.tile_pool(name="scr", bufs=3) as scr,
            tc.tile_pool(name="stat", bufs=1) as stat,
        ):
            slab_t = []
            for j in range(NJ):
                t = slabs.tile([128, SLABW], F32, name=f"slab{j}", tag=f"slab{j}")
                nc.sync.dma_start(t[:], atp[j])
                slab_t.append(t)
            auxt = stat.tile([128, 2 * MT], F32, name="auxt", tag="auxt")
            nc.sync.dma_start(auxt[:], aux.ap())
            ylt = stat.tile([C, SHARD], F32, name="ylt", tag="ylt")
            nc.sync.dma_start(ylt[:], yl.ap())

            # per-(m, group) partial stats: columns m*NG + g
            pos_parts = stat.tile([128, MT * NG], F32, name="ppart", tag="ppart")
            max_parts = stat.tile([128, MT * NG], F32, name="mpart", tag="mpart")
            outt = stat.tile([128, MT], F32, name="outt", tag="outt")

            # bias constants for ACT (float biases need pre-registered const
            # APs, so build [128,1] tiles explicitly)
            bias_c = {}
            for bname, bval in [("m1", -1.0), ("m2", -2.0), ("p2", 2.0), ("z", 0.0)]:
                bt = stat.tile([128, 1], F32, name=f"bc_{bname}", tag=f"bc_{bname}")
                nc.gpsimd.memset(bt[:], bval)
                bias_c[bname] = bt

            for g in range(NG):
                for m in range(MT):
                    pt = psum.tile([128, GW * 512], F32, name="pt", tag="pt")
                    for c in range(KC):
                        for jj in range(GW):
                            j = GW * g + jj
                            lo = c * 512 + m * 128
                            if c < KC - 1:
                                lhsT = slab_t[0][:, lo : lo + 128]
                                rhs = slab_t[j][:, c * 512 : (c + 1) * 512]
                            else:
                                lhsT = ylt[:, m * 128 : (m + 1) * 128]
                                rhs = slab_t[j][0:C, c * 512 : (c + 1) * 512]
                            nc.tensor.matmul(
                                pt[:, jj * 512 : (jj + 1) * 512],
                                lhsT,
                                rhs,
                                start=(c == 0),
                                stop=(c == KC - 1),
                            )
                    idx = m * NG + g
                    t1 = scr.tile([128, GW * 512], F32, name="t1", tag="t1")
                    d1 = scr.tile([128, GW * 512], F32, name="d1", tag="d1")
                    if (g + m) % 2 == 0:
                        # ACT: t1 = relu(-P - 1); d = sqrt(2*t1)
                        nc.scalar.activation(
                            t1[:], pt[:], ACTF.Relu,
                            bias=bias_c["m1"][:], scale=-1.0,
                        )
                        nc.scalar.activation(
                            d1[:],
                            t1[:],
                            ACTF.Sqrt,
                            bias=bias_c["z"][:],
                            scale=2.0,
                            accum_out=pos_parts[:, idx : idx + 1],
                        )
                    else:
                        # DVE: t1 = min(P, -1); d = sqrt(-2*t1 - 2)
                        nc.vector.tensor_scalar(
                            t1[:], pt[:], -1.0, None, op0=ALU.min
                        )
                        nc.scalar.activation(
                            d1[:],
                            t1[:],
                            ACTF.Sqrt,
                            bias=bias_c["m2"][:],
                            scale=-2.0,
                            accum_out=pos_parts[:, idx : idx + 1],
                        )
                    nc.vector.tensor_reduce(
                        max_parts[:, idx : idx + 1], pt[:], axis=AXX, op=ALU.max
                    )

            for m in range(MT):
                ps = stat.tile([128, 1], F32, name=f"ps{m}", tag=f"ps{m}")
                nc.vector.tensor_reduce(
                    ps[:], pos_parts[:, m * NG : (m + 1) * NG], axis=AXX, op=ALU.add
                )
                mx = stat.tile([128, 1], F32, name=f"mx{m}", tag=f"mx{m}")
                nc.vector.tensor_reduce(
                    mx[:], max_parts[:, m * NG : (m + 1) * NG], axis=AXX, op=ALU.max
                )
                # pos_stat = ps / max(cnt,1)  (host passes reciprocal)
                pst = stat.tile([128, 1], F32, name=f"pst{m}", tag=f"pst{m}")
                nc.vector.tensor_scalar(
                    pst[:], ps[:], auxt[:, m : m + 1], None, op0=ALU.mult
                )
                # neg_stat = sqrt(relu(2 - 2*mx))
                nsq = stat.tile([128, 1], F32, name=f"nsq{m}", tag=f"nsq{m}")
                nc.scalar.activation(
                    nsq[:], mx[:], ACTF.Relu, bias=bias_c["p2"][:], scale=-2.0
                )
                ns = stat.tile([128, 1], F32, name=f"ns{m}", tag=f"ns{m}")
                nc.scalar.activation(ns[:], nsq[:], ACTF.Sqrt, bias=bias_c["z"][:])
                # per_row = relu(pos_stat - neg_stat + margin) * valid
                df = stat.tile([128, 1], F32, name=f"df{m}", tag=f"df{m}")
                nc.vector.tensor_tensor(df[:], pst[:], ns[:], op=ALU.subtract)
                pr = stat.tile([128, 1], F32, name=f"pr{m}", tag=f"pr{m}")
                nc.vector.tensor_scalar(
                    pr[:], df[:], MARGIN, 0.0, op0=ALU.add, op1=ALU.max
                )
                nc.vector.tensor_tensor(
                    outt[:, m : m + 1], pr[:], auxt[:, MT + m : MT + m + 1],
                    op=ALU.mult,
                )

            nc.sync.dma_start(outp.ap(), outt[:])

    nc.compile()
    return nc


_NC_CACHE: dict = {}


def _get_nc():
    if "nc" not in _NC_CACHE:
        _NC_CACHE["nc"] = _build_nc()
    return _NC_CACHE["nc"]


def _prep_inputs(embeddings: np.ndarray, labels: np.ndarray):
    E = np.asarray(embeddings, dtype=np.float32)
    L = np.asarray(labels).astype(np.int64)
    assert E.shape == (B, D) and L.shape == (B,)

    nrm = np.maximum(np.linalg.norm(E.astype(np.float32), axis=1), 1e-12)
    N = (E / nrm[:, None].astype(np.float32)).astype(np.float32)

    Y = (L[None, :] == np.arange(C, dtype=np.int64)[:, None]).astype(np.float32)
    s2 = np.float32(np.sqrt(2.0))
    # chunk 4 partitions 0:64 hold -sqrt(2)*Y (the rhs side); the +Y lhsT
    # side ships separately per core (yl).  Partitions 64:128 stay zero.
    AT = np.zeros((KC * 128, B), dtype=np.float32)
    AT[:D] = N.T
    AT[D : D + C] = -s2 * Y

    # slabs[j][p, c*512+x] = AT[128c+p, 512j+x]
    slabs8 = np.ascontiguousarray(
        AT.reshape(KC, 128, NJ, 512).transpose(2, 1, 0, 3).reshape(NJ, 128, SLABW)
    )

    cnt = np.bincount(L, minlength=C)
    pos_cnt = cnt[L] - 1
    neg_cnt = B - cnt[L]
    invc = (1.0 / np.maximum(pos_cnt, 1)).astype(np.float32)
    valid = ((pos_cnt > 0) & (neg_cnt > 0)).astype(np.float32)

    in_maps = []
    for r in range(NCORES):
        rows = slice(SHARD * r, SHARD * (r + 1))
        auxm = np.zeros((128, 2 * MT), dtype=np.float32)
        auxm[:, 0:MT] = invc[rows].reshape(MT, 128).T
        auxm[:, MT : 2 * MT] = valid[rows].reshape(MT, 128).T
        in_maps.append(
            {
                "atp": np.ascontiguousarray(np.roll(slabs8, -r, axis=0)),
                "yl": np.ascontiguousarray(s2 * Y[:, rows]),
                "aux": auxm,
            }
        )
    return in_maps, valid


def _finish(results, valid):
    per_row = np.concatenate(
        [np.asarray(results[r]["outp"]).T.reshape(SHARD) for r in range(NCORES)]
    )
    n_valid = float(valid.sum())
    total = float(per_row.sum(dtype=np.float32))
    out = total / max(n_valid, 1.0) if n_valid > 0 else 0.0
    return np.array(out, dtype=np.float32)


def kernel(embeddings, labels, _run_kwargs=None):
    nc = _get_nc()
    in_maps, valid = _prep_inputs(embeddings, labels)
    res = run_bass_kernel_spmd(
        nc, in_maps, core_ids=list(range(NCORES)), **(_run_kwargs or {})
    )
    out = _finish(res.results, valid)
    if _run_kwargs:
        return out, res
    return out


# revision 11
# speedup vs baseline: 1.8566x; 1.8566x over previous
"""HardTripletLoss on 8 Trainium2 NeuronCores (Bass/Tile).

Math
----
reference: emb = l2_normalize(embeddings); dist = cdist(emb, emb);
  pos_stat[i] = mean_{j: same class, j!=i} dist[i,j]
  neg_stat[i] = min_{j: diff class} dist[i,j]
  loss = mean over valid rows of relu(pos_stat - neg_stat + 1)

For unit vectors dist^2 = 2 - 2*ghat where ghat = N @ N.T.  We fold the
class mask into the GEMM itself: with Y = onehot(labels) [64, B],

  A = [ N.T ; -Y ]  (rhs side; the lhsT +2*Y block ships separately)

and contracting lhsT = [N.T ; +sqrt(2)*Y] (the +Y block from a separate
small per-core tile) against rhs = [N.T ; -sqrt(2)*Y] gives
P = ghat - 2*S  (S = same-class indicator incl diagonal).  Then per row:
  masked positive dists = sqrt(2*relu(-1 - P))   (diff-class & diagonal -> 0)
  hardest negative      = sqrt(relu(2 - 2*rowmax(P)))
      (rowmax(P) = max over diff-class ghat, since same-class P <= -1+eps)

Sharding: rows split 512/core (data parallel).  Every core holds all 4096
columns of A in SBUF (10.5 MB) as 8 slabs of 512 columns; slab order is
rotated per core so each core's first-loaded slab contains its own shard
columns (the matmul stationary operand), letting the GEMM start after the
first 1.3 MB DMA.  Row stats are order-invariant (sum/max over columns).

Host does only input marshaling (normalize+transpose+onehot packing,
O(B*D), 0.02% of the FLOPs) and the final scalar mean over 4096
device-computed per-row losses.
"""

import sys

if "/opt/trn_rl_repo" not in sys.path:
    sys.path.insert(0, "/opt/trn_rl_repo")

import numpy as np

import concourse.bass as bass
import concourse.bacc as bacc
import concourse.mybir as mybir
import concourse.tile as tile
from concourse.bass_utils import run_bass_kernel_spmd

F32 = mybir.dt.float32
F32R = mybir.dt.float32r
ALU = mybir.AluOpType
ACTF = mybir.ActivationFunctionType
AXX = mybir.AxisListType.X

B = 4096
D = 512
C = 64
NCORES = 8
SHARD = B // NCORES          # 512 rows per core
MT = SHARD // 128            # 4 m-tiles per core
NJ = 8                       # column slabs of 512
KC = 5                       # k-chunks of 128 (4 data + 1 +/- onehot)
SLABW = KC * 512             # 2560
GW = 2                       # slabs per psum group ([128, 1024] = 2 banks)
NG = NJ // GW                # 4 psum groups

MARGIN = 1.0


def _build_nc():
    nc = bacc.Bacc(
        "TRN2",
        target_bir_lowering=False,
        debug=False,
        enable_asserts=False,
        num_devices=NCORES,
    )
    atp = nc.dram_tensor("atp", [NJ, 128, SLABW], F32R, kind="ExternalInput")
    yl = nc.dram_tensor("yl", [C, SHARD], F32R, kind="ExternalInput")
    aux = nc.dram_tensor("aux", [128, 2 * MT], F32, kind="ExternalInput")
    outp = nc.dram_tensor("outp", [128, MT], F32, kind="ExternalOutput")

    with tile.TileContext(nc) as tc:
        with (
            tc.tile_pool(name="slabs", bufs=1) as slabs,
            tc.tile_pool(name="psum", bufs=4, space=bass.MemorySpace.PSUM) as psum,
            tc.tile_pool(name="scr", bufs=3) as scr,
            tc.tile_pool(name="stat", bufs=1) as stat,
        ):
            slab_t = []
            for j in range(NJ):
                t = slabs.tile([128, SLABW], F32R, name=f"slab{j}", tag=f"slab{j}")
                nc.sync.dma_start(t[:], atp[j])
                slab_t.append(t)
            auxt = stat.tile([128, 2 * MT], F32, name="auxt", tag="auxt")
            nc.sync.dma_start(auxt[:], aux.ap())
            ylt = stat.tile([C, SHARD], F32R, name="ylt", tag="ylt")
            nc.sync.dma_start(ylt[:], yl.ap())

            # per-(m, group) partial stats: columns m*NG + g
            pos_parts = stat.tile([128, MT * NG], F32, name="ppart", tag="ppart")
            max_parts = stat.tile([128, MT * NG], F32, name="mpart", tag="mpart")
            outt = stat.tile([128, MT], F32, name="outt", tag="outt")

            # bias constants for ACT (float biases need pre-registered const
            # APs, so build [128,1] tiles explicitly)
            bias_c = {}
            for bname, bval in [("m1", -1.0), ("m2", -2.0), ("p2", 2.0), ("z", 0.0)]:
                bt = stat.tile([128, 1], F32, name=f"bc_{bname}", tag=f"bc_{bname}")
                nc.gpsimd.memset(bt[:], bval)
                bias_c[bname] = bt

            for g in range(NG):
                for m in range(MT):
                    pt = psum.tile([128, GW * 512], F32, name="pt", tag="pt")
                    for c in range(KC):
                        for jj in range(GW):
                            j = GW * g + jj
                            lo = c * 512 + m * 128
                            if c < KC - 1:
                                lhsT = slab_t[0][:, lo : lo + 128]
                                rhs = slab_t[j][:, c * 512 : (c + 1) * 512]
                            else:
                                lhsT = ylt[:, m * 128 : (m + 1) * 128]
                                rhs = slab_t[j][0:C, c * 512 : (c + 1) * 512]
                            nc.tensor.matmul(
                                pt[:, jj * 512 : (jj + 1) * 512],
                                lhsT,
                                rhs,
                                start=(c == 0),
                                stop=(c == KC - 1),
                            )
                    idx = m * NG + g
                    t1 = scr.tile([128, GW * 512], F32, name="t1", tag="t1")
                    d1 = scr.tile([128, GW * 512], F32, name="d1", tag="d1")
                    if (g + m) % 2 == 0:
                        # ACT: t1 = relu(-P - 1); d = sqrt(2*t1)
                        nc.scalar.activation(
                            t1[:], pt[:], ACTF.Relu,
                            bias=bias_c["m1"][:], scale=-1.0,
                        )
                        nc.scalar.activation(
                            d1[:],
                            t1[:],
                            ACTF.Sqrt,
                            bias=bias_c["z"][:],
                            scale=2.0,
                            accum_out=pos_parts[:, idx : idx + 1],
                        )
                    else:
                        # DVE: t1 = min(P, -1); d = sqrt(-2*t1 - 2)
                        nc.vector.tensor_scalar(
                            t1[:], pt[:], -1.0, None, op0=ALU.min
                        )
                        nc.scalar.activation(
                            d1[:],
                            t1[:],
                            ACTF.Sqrt,
                            bias=bias_c["m2"][:],
                            scale=-2.0,
                            accum_out=pos_parts[:, idx : idx + 1],
                        )
                    nc.vector.tensor_reduce(
                        max_parts[:, idx : idx + 1], pt[:], axis=AXX, op=ALU.max
                    )

            for m in range(MT):
                ps = stat.tile([128, 1], F32, name=f"ps{m}", tag=f"ps{m}")
                nc.vector.tensor_reduce(
                    ps[:], pos_parts[:, m * NG : (m + 1) * NG], axis=AXX, op=ALU.add
                )
                mx = stat.tile([128, 1], F32, name=f"mx{m}", tag=f"mx{m}")
                nc.vector.tensor_reduce(
                    mx[:], max_parts[:, m * NG : (m + 1) * NG], axis=AXX, op=ALU.max
                )
                # pos_stat = ps / max(cnt,1)  (host passes reciprocal)
                pst = stat.tile([128, 1], F32, name=f"pst{m}", tag=f"pst{m}")
                nc.vector.tensor_scalar(
                    pst[:], ps[:], auxt[:, m : m + 1], None, op0=ALU.mult
                )
                # neg_stat = sqrt(relu(2 - 2*mx))
                nsq = stat.tile([128, 1], F32, name=f"nsq{m}", tag=f"nsq{m}")
                nc.scalar.activation(
                    nsq[:], mx[:], ACTF.Relu, bias=bias_c["p2"][:], scale=-2.0
                )
                ns = stat.tile([128, 1], F32, name=f"ns{m}", tag=f"ns{m}")
                nc.scalar.activation(ns[:], nsq[:], ACTF.Sqrt, bias=bias_c["z"][:])
                # per_row = relu(pos_stat - neg_stat + margin) * valid
                df = stat.tile([128, 1], F32, name=f"df{m}", tag=f"df{m}")
                nc.vector.tensor_tensor(df[:], pst[:], ns[:], op=ALU.subtract)
                pr = stat.tile([128, 1], F32, name=f"pr{m}", tag=f"pr{m}")
                nc.vector.tensor_scalar(
                    pr[:], df[:], MARGIN, 0.0, op0=ALU.add, op1=ALU.max
                )
                nc.vector.tensor_tensor(
                    outt[:, m : m + 1], pr[:], auxt[:, MT + m : MT + m + 1],
                    op=ALU.mult,
                )

            nc.sync.dma_start(outp.ap(), outt[:])

    nc.compile()
    return nc


_NC_CACHE: dict = {}


def _get_nc():
    if "nc" not in _NC_CACHE:
        _NC_CACHE["nc"] = _build_nc()
    return _NC_CACHE["nc"]


def _prep_inputs(embeddings: np.ndarray, labels: np.ndarray):
    E = np.asarray(embeddings, dtype=np.float32)
    L = np.asarray(labels).astype(np.int64)
    assert E.shape == (B, D) and L.shape == (B,)

    nrm = np.maximum(np.linalg.norm(E.astype(np.float32), axis=1), 1e-12)
    N = (E / nrm[:, None].astype(np.float32)).astype(np.float32)

    Y = (L[None, :] == np.arange(C, dtype=np.int64)[:, None]).astype(np.float32)
    # chunk 4 partitions 0:64 hold -Y (the rhs side); the +2*Y lhsT side
    # ships separately per core (yl).  Partitions 64:128 stay zero.
    AT = np.zeros((KC * 128, B), dtype=np.float32)
    AT[:D] = N.T
    AT[D : D + C] = -Y

    # slabs[j][p, c*512+x] = AT[128c+p, 512j+x]
    slabs8 = np.ascontiguousarray(
        AT.reshape(KC, 128, NJ, 512).transpose(2, 1, 0, 3).reshape(NJ, 128, SLABW)
    )

    cnt = np.bincount(L, minlength=C)
    pos_cnt = cnt[L] - 1
    neg_cnt = B - cnt[L]
    invc = (1.0 / np.maximum(pos_cnt, 1)).astype(np.float32)
    valid = ((pos_cnt > 0) & (neg_cnt > 0)).astype(np.float32)

    in_maps = []
    for r in range(NCORES):
        rows = slice(SHARD * r, SHARD * (r + 1))
        auxm = np.zeros((128, 2 * MT), dtype=np.float32)
        auxm[:, 0:MT] = invc[rows].reshape(MT, 128).T
        auxm[:, MT : 2 * MT] = valid[rows].reshape(MT, 128).T
        in_maps.append(
            {
                "atp": np.ascontiguousarray(np.roll(slabs8, -r, axis=0)),
                "yl": np.ascontiguousarray(2.0 * Y[:, rows]),
                "aux": auxm,
            }
        )
    return in_maps, valid


def _finish(results, valid):
    per_row = np.concatenate(
        [np.asarray(results[r]["outp"]).T.reshape(SHARD) for r in range(NCORES)]
    )
    n_valid = float(valid.sum())
    total = float(per_row.sum(dtype=np.float32))
    out = total / max(n_valid, 1.0) if n_valid > 0 else 0.0
    return np.array(out, dtype=np.float32)


def kernel(embeddings, labels, _run_kwargs=None):
    nc = _get_nc()
    in_maps, valid = _prep_inputs(embeddings, labels)
    res = run_bass_kernel_spmd(
        nc, in_maps, core_ids=list(range(NCORES)), **(_run_kwargs or {})
    )
    out = _finish(res.results, valid)
    if _run_kwargs:
        return out, res
    return out


# revision 12
# speedup vs baseline: 2.4328x; 1.3104x over previous
"""HardTripletLoss on 8 Trainium2 NeuronCores (Bass/Tile).

Math
----
reference: emb = l2_normalize(embeddings); dist = cdist(emb, emb);
  pos_stat[i] = mean_{j: same class, j!=i} dist[i,j]
  neg_stat[i] = min_{j: diff class} dist[i,j]
  loss = mean over valid rows of relu(pos_stat - neg_stat + 1)

For unit vectors dist^2 = 2 - 2*ghat where ghat = N @ N.T.  We fold the
class mask into the GEMM itself: with Y = onehot(labels) [64, B],

  A = [ N.T ; -Y ]  (rhs side; the lhsT +2*Y block ships separately)

and contracting lhsT = [N.T ; +sqrt(2)*Y] (the +Y block from a separate
small per-core tile) against rhs = [N.T ; -sqrt(2)*Y] gives
P = ghat - 2*S  (S = same-class indicator incl diagonal).  Then per row:
  masked positive dists = sqrt(2*relu(-1 - P))   (diff-class & diagonal -> 0)
  hardest negative      = sqrt(relu(2 - 2*rowmax(P)))
      (rowmax(P) = max over diff-class ghat, since same-class P <= -1+eps)

Sharding: rows split 512/core (data parallel).  Every core holds all 4096
columns of A in SBUF (10.5 MB) as 8 slabs of 512 columns; slab order is
rotated per core so each core's first-loaded slab contains its own shard
columns (the matmul stationary operand), letting the GEMM start after the
first 1.3 MB DMA.  Row stats are order-invariant (sum/max over columns).

Host does only input marshaling (normalize+transpose+onehot packing,
O(B*D), 0.02% of the FLOPs) and the final scalar mean over 4096
device-computed per-row losses.
"""

import sys

if "/opt/trn_rl_repo" not in sys.path:
    sys.path.insert(0, "/opt/trn_rl_repo")

import numpy as np

import concourse.bass as bass
import concourse.bacc as bacc
import concourse.mybir as mybir
import concourse.tile as tile
from concourse.bass_utils import run_bass_kernel_spmd

F32 = mybir.dt.float32
F32R = mybir.dt.float32r
ALU = mybir.AluOpType
ACTF = mybir.ActivationFunctionType
AXX = mybir.AxisListType.X

B = 4096
D = 512
C = 64
NCORES = 8
SHARD = B // NCORES          # 512 rows per core
MT = SHARD // 128            # 4 m-tiles per core
NJ = 8                       # column slabs of 512
KC = 5                       # k-chunks of 128 (4 data + 1 +/- onehot)
SLABW = KC * 512             # 2560
GW = 2                       # slabs per psum group ([128, 1024] = 2 banks)
NG = NJ // GW                # 4 psum groups

MARGIN = 1.0


def _build_nc():
    nc = bacc.Bacc(
        "TRN2",
        target_bir_lowering=False,
        debug=False,
        enable_asserts=False,
        num_devices=NCORES,
    )
    atp = nc.dram_tensor("atp", [NJ, 128, SLABW], F32R, kind="ExternalInput")
    yl = nc.dram_tensor("yl", [C, SHARD], F32R, kind="ExternalInput")
    pos_d = nc.dram_tensor("pos", [128, MT * NG], F32, kind="ExternalOutput")
    mxp_d = nc.dram_tensor("mxp", [128, MT * NG], F32, kind="ExternalOutput")

    with tile.TileContext(nc) as tc:
        with (
            tc.tile_pool(name="slabs", bufs=1) as slabs,
            tc.tile_pool(name="psum", bufs=4, space=bass.MemorySpace.PSUM) as psum,
            tc.tile_pool(name="scr", bufs=3) as scr,
            tc.tile_pool(name="stat", bufs=1) as stat,
        ):
            # small lhsT-side one-hot block first: every group's c=4 matmul
            # needs it, so it must not queue behind 10 MB of slab DMA
            ylt = stat.tile([C, SHARD], F32R, name="ylt", tag="ylt")
            nc.sync.dma_start(ylt[:], yl.ap())
            slab_t = []
            for j in range(NJ):
                t = slabs.tile([128, SLABW], F32R, name=f"slab{j}", tag=f"slab{j}")
                nc.sync.dma_start(t[:], atp[j])
                slab_t.append(t)

            # per-(m, group) partial stats: columns m*NG + g
            pos_parts = stat.tile([128, MT * NG], F32, name="ppart", tag="ppart")
            max_parts = stat.tile([128, MT * NG], F32, name="mpart", tag="mpart")

            # bias constants for ACT (float biases need pre-registered const
            # APs, so build [128,1] tiles explicitly)
            bias_c = {}
            for bname, bval in [("m1", -1.0), ("m2", -2.0), ("z", 0.0)]:
                bt = stat.tile([128, 1], F32, name=f"bc_{bname}", tag=f"bc_{bname}")
                nc.gpsimd.memset(bt[:], bval)
                bias_c[bname] = bt

            for g in range(NG):
                for m in range(MT):
                    pt = psum.tile([128, GW * 512], F32, name="pt", tag="pt")
                    for c in range(KC):
                        for jj in range(GW):
                            j = GW * g + jj
                            lo = c * 512 + m * 128
                            if c < KC - 1:
                                lhsT = slab_t[0][:, lo : lo + 128]
                                rhs = slab_t[j][:, c * 512 : (c + 1) * 512]
                            else:
                                lhsT = ylt[:, m * 128 : (m + 1) * 128]
                                rhs = slab_t[j][0:C, c * 512 : (c + 1) * 512]
                            nc.tensor.matmul(
                                pt[:, jj * 512 : (jj + 1) * 512],
                                lhsT,
                                rhs,
                                start=(c == 0),
                                stop=(c == KC - 1),
                            )
                    idx = m * NG + g
                    t1 = scr.tile([128, GW * 512], F32, name="t1", tag="t1")
                    d1 = scr.tile([128, GW * 512], F32, name="d1", tag="d1")
                    if (g + m) % 2 == 0:
                        # ACT: t1 = relu(-P - 1); d = sqrt(2*t1)
                        nc.scalar.activation(
                            t1[:], pt[:], ACTF.Relu,
                            bias=bias_c["m1"][:], scale=-1.0,
                        )
                        nc.scalar.activation(
                            d1[:],
                            t1[:],
                            ACTF.Sqrt,
                            bias=bias_c["z"][:],
                            scale=2.0,
                            accum_out=pos_parts[:, idx : idx + 1],
                        )
                    else:
                        # DVE: t1 = min(P, -1); d = sqrt(-2*t1 - 2)
                        nc.vector.tensor_scalar(
                            t1[:], pt[:], -1.0, None, op0=ALU.min
                        )
                        nc.scalar.activation(
                            d1[:],
                            t1[:],
                            ACTF.Sqrt,
                            bias=bias_c["m2"][:],
                            scale=-2.0,
                            accum_out=pos_parts[:, idx : idx + 1],
                        )
                    nc.vector.tensor_reduce(
                        max_parts[:, idx : idx + 1], pt[:], axis=AXX, op=ALU.max
                    )

            nc.sync.dma_start(pos_d.ap(), pos_parts[:])
            nc.sync.dma_start(mxp_d.ap(), max_parts[:])

    nc.compile()
    return nc


_NC_CACHE: dict = {}


def _get_nc():
    if "nc" not in _NC_CACHE:
        _NC_CACHE["nc"] = _build_nc()
    return _NC_CACHE["nc"]


def _prep_inputs(embeddings: np.ndarray, labels: np.ndarray):
    E = np.asarray(embeddings, dtype=np.float32)
    L = np.asarray(labels).astype(np.int64)
    assert E.shape == (B, D) and L.shape == (B,)

    nrm = np.maximum(np.linalg.norm(E.astype(np.float32), axis=1), 1e-12)
    N = (E / nrm[:, None].astype(np.float32)).astype(np.float32)

    Y = (L[None, :] == np.arange(C, dtype=np.int64)[:, None]).astype(np.float32)
    # chunk 4 partitions 0:64 hold -Y (the rhs side); the +2*Y lhsT side
    # ships separately per core (yl).  Partitions 64:128 stay zero.
    AT = np.zeros((KC * 128, B), dtype=np.float32)
    AT[:D] = N.T
    AT[D : D + C] = -Y

    # slabs[j][p, c*512+x] = AT[128c+p, 512j+x]
    slabs8 = np.ascontiguousarray(
        AT.reshape(KC, 128, NJ, 512).transpose(2, 1, 0, 3).reshape(NJ, 128, SLABW)
    )

    cnt = np.bincount(L, minlength=C)
    pos_cnt = cnt[L] - 1
    neg_cnt = B - cnt[L]
    invc = (1.0 / np.maximum(pos_cnt, 1)).astype(np.float32)
    valid = ((pos_cnt > 0) & (neg_cnt > 0)).astype(np.float32)

    in_maps = []
    for r in range(NCORES):
        rows = slice(SHARD * r, SHARD * (r + 1))
        in_maps.append(
            {
                "atp": np.ascontiguousarray(np.roll(slabs8, -r, axis=0)),
                "yl": np.ascontiguousarray(2.0 * Y[:, rows]),
            }
        )
    return in_maps, (invc, valid)


def _finish(results, aux):
    invc, valid = aux
    pos_sum = np.empty(B, dtype=np.float32)
    max_p = np.empty(B, dtype=np.float32)
    for r in range(NCORES):
        # [128, MT*NG] -> per (m, g) columns; row index = 512*r + 128*m + p
        pp = np.asarray(results[r]["pos"]).reshape(128, MT, NG)
        mp = np.asarray(results[r]["mxp"]).reshape(128, MT, NG)
        rows = slice(SHARD * r, SHARD * (r + 1))
        pos_sum[rows] = pp.sum(axis=2, dtype=np.float32).T.reshape(SHARD)
        max_p[rows] = mp.max(axis=2).T.reshape(SHARD)
    pos_stat = pos_sum * invc
    neg_stat = np.sqrt(np.maximum(2.0 - 2.0 * max_p, 0.0), dtype=np.float32)
    per_row = np.maximum(pos_stat - neg_stat + MARGIN, 0.0) * valid
    n_valid = float(valid.sum())
    total = float(per_row.sum(dtype=np.float32))
    out = total / max(n_valid, 1.0) if n_valid > 0 else 0.0
    return np.array(out, dtype=np.float32)


def kernel(embeddings, labels, _run_kwargs=None):
    nc = _get_nc()
    in_maps, aux = _prep_inputs(embeddings, labels)
    res = run_bass_kernel_spmd(
        nc, in_maps, core_ids=list(range(NCORES)), **(_run_kwargs or {})
    )
    out = _finish(res.results, aux)
    if _run_kwargs:
        return out, res
    return out


# revision 13
# speedup vs baseline: 2.7061x; 1.1123x over previous
"""HardTripletLoss on 8 Trainium2 NeuronCores (Bass/Tile).

Math
----
reference: emb = l2_normalize(embeddings); dist = cdist(emb, emb);
  pos_stat[i] = mean_{j: same class, j!=i} dist[i,j]
  neg_stat[i] = min_{j: diff class} dist[i,j]
  loss = mean over valid rows of relu(pos_stat - neg_stat + 1)

For unit vectors dist^2 = 2 - 2*ghat where ghat = N @ N.T.  We fold the
class mask into the GEMM itself: with Y = onehot(labels) [64, B],

  A = [ N.T ; -Y ]  (rhs side; the lhsT +2*Y block ships separately)

and contracting lhsT = [N.T ; +sqrt(2)*Y] (the +Y block from a separate
small per-core tile) against rhs = [N.T ; -sqrt(2)*Y] gives
P = ghat - 2*S  (S = same-class indicator incl diagonal).  Then per row:
  masked positive dists = sqrt(2*relu(-1 - P))   (diff-class & diagonal -> 0)
  hardest negative      = sqrt(relu(2 - 2*rowmax(P)))
      (rowmax(P) = max over diff-class ghat, since same-class P <= -1+eps)

Sharding: rows split 512/core (data parallel).  Every core holds all 4096
columns of A in SBUF (10.5 MB) as 8 slabs of 512 columns; slab order is
rotated per core so each core's first-loaded slab contains its own shard
columns (the matmul stationary operand), letting the GEMM start after the
first 1.3 MB DMA.  Row stats are order-invariant (sum/max over columns).

Host does only input marshaling (normalize+transpose+onehot packing,
O(B*D), 0.02% of the FLOPs) and the final scalar mean over 4096
device-computed per-row losses.
"""

import sys

if "/opt/trn_rl_repo" not in sys.path:
    sys.path.insert(0, "/opt/trn_rl_repo")

import ml_dtypes
import numpy as np


import concourse.bass as bass
import concourse.bacc as bacc
import concourse.mybir as mybir
import concourse.tile as tile
from concourse.bass_utils import run_bass_kernel_spmd

F32 = mybir.dt.float32
F32R = mybir.dt.float32r
BF16 = mybir.dt.bfloat16
GEMM_DT = BF16  # bf16: fast weight load + half DMA; f32r fallback if accuracy demands
ALU = mybir.AluOpType
ACTF = mybir.ActivationFunctionType
AXX = mybir.AxisListType.X

B = 4096
D = 512
C = 64
NCORES = 8
SHARD = B // NCORES          # 512 rows per core
MT = SHARD // 128            # 4 m-tiles per core
NJ = 8                       # column slabs of 512
KC = 5                       # k-chunks of 128 (4 data + 1 +/- onehot)
SLABW = KC * 512             # 2560
GW = 2                       # slabs per psum group ([128, 1024] = 2 banks)
NG = NJ // GW                # 4 psum groups

MARGIN = 1.0


def _build_nc():
    nc = bacc.Bacc(
        "TRN2",
        target_bir_lowering=False,
        debug=False,
        enable_asserts=False,
        num_devices=NCORES,
    )
    atp = nc.dram_tensor("atp", [NJ, 128, SLABW], GEMM_DT, kind="ExternalInput")
    yl = nc.dram_tensor("yl", [C, SHARD], GEMM_DT, kind="ExternalInput")
    pos_d = nc.dram_tensor("pos", [128, MT * NG], F32, kind="ExternalOutput")
    mxp_d = nc.dram_tensor("mxp", [128, MT * NG], F32, kind="ExternalOutput")

    with tile.TileContext(nc) as tc:
        with (
            tc.tile_pool(name="slabs", bufs=1) as slabs,
            tc.tile_pool(name="psum", bufs=4, space=bass.MemorySpace.PSUM) as psum,
            tc.tile_pool(name="scr", bufs=3) as scr,
            tc.tile_pool(name="stat", bufs=1) as stat,
        ):
            # small lhsT-side one-hot block first: every group's c=4 matmul
            # needs it, so it must not queue behind 10 MB of slab DMA
            ylt = stat.tile([C, SHARD], GEMM_DT, name="ylt", tag="ylt")
            nc.sync.dma_start(ylt[:], yl.ap())
            slab_t = []
            for j in range(NJ):
                t = slabs.tile([128, SLABW], GEMM_DT, name=f"slab{j}", tag=f"slab{j}")
                nc.sync.dma_start(t[:], atp[j])
                slab_t.append(t)

            # per-(m, group) partial stats: columns m*NG + g
            pos_parts = stat.tile([128, MT * NG], F32, name="ppart", tag="ppart")
            max_parts = stat.tile([128, MT * NG], F32, name="mpart", tag="mpart")

            # bias constants for ACT (float biases need pre-registered const
            # APs, so build [128,1] tiles explicitly)
            bias_c = {}
            for bname, bval in [("m1", -1.0), ("m2", -2.0), ("z", 0.0)]:
                bt = stat.tile([128, 1], F32, name=f"bc_{bname}", tag=f"bc_{bname}")
                nc.gpsimd.memset(bt[:], bval)
                bias_c[bname] = bt

            warm = stat.tile([128, 1], F32, name="warm", tag="warm")
            nc.scalar.activation(warm[:], bias_c["z"][:], ACTF.Relu)
            nc.scalar.activation(warm[:], warm[:], ACTF.Sqrt, bias=bias_c["z"][:])

            for g in range(NG):
                for m in range(MT):
                    pt = psum.tile([128, GW * 512], F32, name="pt", tag="pt")
                    for c in range(KC):
                        for jj in range(GW):
                            j = GW * g + jj
                            lo = c * 512 + m * 128
                            if c < KC - 1:
                                lhsT = slab_t[0][:, lo : lo + 128]
                                rhs = slab_t[j][:, c * 512 : (c + 1) * 512]
                            else:
                                lhsT = ylt[:, m * 128 : (m + 1) * 128]
                                rhs = slab_t[j][0:C, c * 512 : (c + 1) * 512]
                            nc.tensor.matmul(
                                pt[:, jj * 512 : (jj + 1) * 512],
                                lhsT,
                                rhs,
                                start=(c == 0),
                                stop=(c == KC - 1),
                            )
                    idx = m * NG + g
                    t1 = scr.tile([128, GW * 512], F32, name="t1", tag="t1")
                    d1 = scr.tile([128, GW * 512], F32, name="d1", tag="d1")
                    if (g + m) % 2 == 0:
                        # ACT: t1 = relu(-P - 1); d = sqrt(2*t1)
                        nc.scalar.activation(
                            t1[:], pt[:], ACTF.Relu,
                            bias=bias_c["m1"][:], scale=-1.0,
                        )
                        nc.scalar.activation(
                            d1[:],
                            t1[:],
                            ACTF.Sqrt,
                            bias=bias_c["z"][:],
                            scale=2.0,
                            accum_out=pos_parts[:, idx : idx + 1],
                        )
                    else:
                        # DVE: t1 = min(P, -1); d = sqrt(-2*t1 - 2)
                        nc.vector.tensor_scalar(
                            t1[:], pt[:], -1.0, None, op0=ALU.min
                        )
                        nc.scalar.activation(
                            d1[:],
                            t1[:],
                            ACTF.Sqrt,
                            bias=bias_c["m2"][:],
                            scale=-2.0,
                            accum_out=pos_parts[:, idx : idx + 1],
                        )
                    nc.vector.tensor_reduce(
                        max_parts[:, idx : idx + 1], pt[:], axis=AXX, op=ALU.max
                    )

            nc.sync.dma_start(pos_d.ap(), pos_parts[:])
            nc.sync.dma_start(mxp_d.ap(), max_parts[:])

    nc.compile()
    return nc


_NC_CACHE: dict = {}


def _get_nc():
    if "nc" not in _NC_CACHE:
        _NC_CACHE["nc"] = _build_nc()
    return _NC_CACHE["nc"]


def _prep_inputs(embeddings: np.ndarray, labels: np.ndarray):
    E = np.asarray(embeddings, dtype=np.float32)
    L = np.asarray(labels).astype(np.int64)
    assert E.shape == (B, D) and L.shape == (B,)

    nrm = np.maximum(np.linalg.norm(E.astype(np.float32), axis=1), 1e-12)
    N = (E / nrm[:, None].astype(np.float32)).astype(np.float32)

    Y = (L[None, :] == np.arange(C, dtype=np.int64)[:, None]).astype(np.float32)
    # chunk 4 partitions 0:64 hold -Y (the rhs side); the +2*Y lhsT side
    # ships separately per core (yl).  Partitions 64:128 stay zero.
    AT = np.zeros((KC * 128, B), dtype=np.float32)
    AT[:D] = N.T
    AT[D : D + C] = -Y

    # slabs[j][p, c*512+x] = AT[128c+p, 512j+x]
    slabs8 = np.ascontiguousarray(
        AT.reshape(KC, 128, NJ, 512)
        .transpose(2, 1, 0, 3)
        .reshape(NJ, 128, SLABW)
        .astype(ml_dtypes.bfloat16)
    )

    cnt = np.bincount(L, minlength=C)
    pos_cnt = cnt[L] - 1
    neg_cnt = B - cnt[L]
    invc = (1.0 / np.maximum(pos_cnt, 1)).astype(np.float32)
    valid = ((pos_cnt > 0) & (neg_cnt > 0)).astype(np.float32)

    in_maps = []
    for r in range(NCORES):
        rows = slice(SHARD * r, SHARD * (r + 1))
        in_maps.append(
            {
                "atp": np.ascontiguousarray(np.roll(slabs8, -r, axis=0)),
                "yl": np.ascontiguousarray((2.0 * Y[:, rows]).astype(ml_dtypes.bfloat16)),
            }
        )
    return in_maps, (invc, valid)


def _finish(results, aux):
    invc, valid = aux
    pos_sum = np.empty(B, dtype=np.float32)
    max_p = np.empty(B, dtype=np.float32)
    for r in range(NCORES):
        # [128, MT*NG] -> per (m, g) columns; row index = 512*r + 128*m + p
        pp = np.asarray(results[r]["pos"]).reshape(128, MT, NG)
        mp = np.asarray(results[r]["mxp"]).reshape(128, MT, NG)
        rows = slice(SHARD * r, SHARD * (r + 1))
        pos_sum[rows] = pp.sum(axis=2, dtype=np.float32).T.reshape(SHARD)
        max_p[rows] = mp.max(axis=2).T.reshape(SHARD)
    pos_stat = pos_sum * invc
    neg_stat = np.sqrt(np.maximum(2.0 - 2.0 * max_p, 0.0), dtype=np.float32)
    per_row = np.maximum(pos_stat - neg_stat + MARGIN, 0.0) * valid
    n_valid = float(valid.sum())
    total = float(per_row.sum(dtype=np.float32))
    out = total / max(n_valid, 1.0) if n_valid > 0 else 0.0
    return np.array(out, dtype=np.float32)


def kernel(embeddings, labels, _run_kwargs=None):
    nc = _get_nc()
    in_maps, aux = _prep_inputs(embeddings, labels)
    res = run_bass_kernel_spmd(
        nc, in_maps, core_ids=list(range(NCORES)), **(_run_kwargs or {})
    )
    out = _finish(res.results, aux)
    if _run_kwargs:
        return out, res
    return out
